# revision 17
# baseline (speedup 1.0000x reference)
"""Trainium2 Bass kernel for CenterDependentPool2D.

Input  x: (8, 64, 448, 448) fp32  ->  Output: (8, 64, 224, 224) fp32.

Strategy (per core = one batch element, 64 channels):
  - Partition p = c + 64*wg: channel c, wg = column half (0: out cols 0..111,
    1: out cols 112..223). Both pooling axes live in the free dimension.
  - Host pre-packs the input per-partition as fp16 with NEG pads baked in
    (xprep [128, 448, 250]); DMA descriptors are 16KB contiguous chunks.
    Output is staged per-partition fp16 (yprep [128, 224, 112]) and
    reassembled + cast to fp32 on the host.
  - All five ring windows (k in {2,8,14,20,26}, stride 2, reflect pad==clip)
    decompose over pair-max arrays E/O in both axes; ring r's window is an
    s x s stride-1 square over EE (s=1/7/13) or OO (s=4/10) computed with a
    shared shifted-max pyramid.
  - Ring-dependent pyramid stages are column-gated: each ring occupies a
    narrow radial interval per 16-row band, which is a single column interval
    per half. Ops are emitted per half (64 partitions) over exactly the hull
    columns needed; DVE time is FD-proportional regardless of partition
    count, so narrow per-half ops are pure savings. Wide unions fall back to
    a single 128-partition op.
  - Blend uses exact (disjoint) annulus masks with copy_predicated over ring
    hulls only; s13 (outer ring) is written unconditionally over its hull.
  - owt/oo construction runs on GPSIMD; rolling-tile copies on ScalarE.
"""

import numpy as np

import concourse.bass as bass
import concourse.mybir as mybir
from concourse.tile import TileContext
from concourse.bass_utils import run_bass_kernel_spmd

# ---------------- problem constants ----------------
B, C, IN, OUT = 8, 64, 448, 224
OW = 112          # out cols per half
EW = 124          # e-column count of pair arrays
WIN = 250         # input tile cols (incl pads)
NEG = -30000.0    # "minus infinity" that survives fp16
RADII = (60, 75, 90, 105)

DT = mybir.dt.float16
WIDE = 116        # per-half emission threshold: wL+wR above this -> single op

_CACHED = {}


# ---------------- static ring geometry ----------------
def _ring_masks():
    yy, xx = np.mgrid[0:OUT, 0:OUT]
    d2 = (yy - OUT // 2) ** 2 + (xx - OUT // 2) ** 2
    d = {r: d2 < r * r for r in RADII}
    r20 = d[105] & ~d[90]
    r14 = d[90] & ~d[75]
    r8 = d[75] & ~d[60]
    r2 = d[60]
    outer = ~d[105]
    return [r20, r14, r8, r2], outer


RINGS, OUTER = _ring_masks()


def _hulls(mask, y0, y1):
    """Per-half inclusive col interval [(lo,hi)|None left, ... right] of mask
    rows y0:y1. Left half = out cols 0..111 (local=col), right = 112..223
    (local=col-112)."""
    out = []
    for wg in range(2):
        cols = mask[y0:y1, wg * OW:(wg + 1) * OW].any(axis=0)
        nz = np.flatnonzero(cols)
        out.append((int(nz[0]), int(nz[-1])) if len(nz) else None)
    return out


def _hull2(a, b):
    if a is None:
        return b
    if b is None:
        return a
    return (min(a[0], b[0]), max(a[1], b[1]))


def _shift(iv, lo_d, hi_d, hi_cap=EW - 1):
    if iv is None:
        return None
    return (iv[0] + lo_d, min(iv[1] + hi_d, hi_cap))


def _band_plan(y0, y1):
    """All per-half need-intervals for one band."""
    h26 = _hulls(OUTER, y0, y1)
    h20 = _hulls(RINGS[0], y0, y1)
    h14 = _hulls(RINGS[1], y0, y1)
    h8 = _hulls(RINGS[2], y0, y1)
    h2 = _hulls(RINGS[3], y0, y1)
    p = {"h26": h26, "h20": h20, "h14": h14, "h8": h8, "h2": h2}
    # EE side needs (per half)
    p["s13"] = h26
    p["v"] = [_shift(h, 0, 5) for h in h26]          # also = s8 need
    p["a8"] = [_shift(h, 0, 9) for h in h26]         # = s4t cols via rows op
    p["s7"] = [_shift(h, 3, 3) for h in h14]
    p["u"] = [_shift(h, 3, 6) for h in h14]          # = s4t cols for k14
    p["s4t"] = [_hull2(a, b) for a, b in zip(p["a8"], p["u"])]
    p["a4"] = [_shift(h, 0, 2) for h in p["s4t"]]    # = s2 need
    p["a2"] = [_shift(h, 0, 1) for h in p["a4"]]     # = ee need (pyramid)
    p["ee"] = [_hull2(a, _shift(b, 6, 6)) for a, b in zip(p["a2"], h2)]
    # OO side needs
    p["s10"] = [_shift(h, 1, 1) for h in h20]
    p["w"] = [_shift(h, 1, 3) for h in h20]          # = s8o need
    p["a8o"] = [_shift(h, 1, 7) for h in h20]        # = s4o cols for k20
    p["k8v"] = [_shift(h, 4, 4) for h in h8]         # s4o cols used by blend
    p["s4o"] = [_hull2(a, b) for a, b in zip(p["a8o"], p["k8v"])]
    p["a4o"] = [_shift(h, 0, 2) for h in p["s4o"]]   # = s2o need
    p["a2o"] = [_shift(h, 0, 1) for h in p["a4o"]]   # = oo need
    return p


def _build_masks() -> np.ndarray:
    """RMASK [128, 4, 224, 112] u8: exact annulus masks per partition.
    ridx 0..3 = ring k20, k14, k8, k2 (disjoint)."""
    rings = np.stack([m.astype(np.uint8) for m in RINGS])  # [4, 224, 224]
    rm = np.zeros((128, 4, OUT, OW), np.uint8)
    for p in range(128):
        wg = p // 64
        rm[p] = rings[:, :, wg * OW:(wg + 1) * OW]
    return rm


def split_multi_waits(nc):
    """walrus CoreV3Gen accepts at most 1 sync-wait per instruction; Tile's
    tail drains can carry 2+.  Peel extras onto preceding NoOps."""
    n = 0
    for fn in nc.m.functions:
        for bb in fn.blocks:
            insts = list(bb.instructions)
            out = []
            for ins in insts:
                si = getattr(ins, "sync_info", None)
                if si is not None and len(si.on_wait) > 1:
                    waits = list(si.on_wait)
                    for k, w in enumerate(waits[:-1]):
                        nop = mybir.InstNoOp(
                            name=f"{ins.name}-waitsplit{k}",
                            engine=ins.engine, ins=[], outs=[])
                        nop.sync_info = mybir.SyncInfo(on_wait=[w], on_update=[])
                        out.append(nop)
                        n += 1
                    ins.sync_info = mybir.SyncInfo(
                        on_wait=[waits[-1]], on_update=list(si.on_update))
                out.append(ins)
            if n:
                bb.instructions = out
    return n


def _emit_kernel(nc: bass.Bass):
    x = nc.dram_tensor("x", [128, IN, WIN], DT, kind="ExternalInput")
    y = nc.dram_tensor("y", [128, OUT, OW], DT, kind="ExternalOutput")
    rmask = nc.inline_tensor(_build_masks(), name="rmask")

    dve = nc.vector
    gps = nc.vector   # GPSIMD TensorTensor fails CoreV3 ISA check
    act = nc.scalar
    mx = mybir.AluOpType.max

    with TileContext(nc) as tc:
        with tc.tile_pool(name="pp", bufs=1) as pers, \
             tc.tile_pool(name="tp", bufs=2) as tP, \
             tc.tile_pool(name="tq", bufs=2) as tQ, \
             tc.tile_pool(name="tr", bufs=3) as tR, \
             tc.tile_pool(name="to", bufs=2) as tPo, \
             tc.tile_pool(name="tqo", bufs=2) as tQo, \
             tc.tile_pool(name="tro", bufs=2) as tRo:

            it_bufs = [pers.tile([128, 32, WIN], DT, tag=f"in{i}",
                                 name=f"itile{i}") for i in range(2)]
            ewt = pers.tile([128, 60, EW], DT, tag="ewt")
            owt = pers.tile([128, 60, EW], DT, tag="owt")
            ee = pers.tile([128, 28, EW], DT, tag="ee")
            oo = pers.tile([128, 28, EW], DT, tag="oo")
            s2t = pers.tile([128, 27, EW], DT, tag="s2")
            s4t = pers.tile([128, 25, EW], DT, tag="s4")
            s2ot = pers.tile([128, 24, EW], DT, tag="s2o")
            s4o = pers.tile([128, 22, EW], DT, tag="s4o")
            mask_t = pers.tile([128, 4, 16, OW], mybir.dt.uint8, tag="mk")

            def emit(fn, ivs, pad=0):
                """Emit fn(p0, p1, lo, hi) per half over inclusive intervals
                ivs=[left,right], merging into one 128-part op when wide."""
                l, r = ivs
                if l is None and r is None:
                    return
                if l is not None and r is not None:
                    if (l[1] - l[0] + 1) + (r[1] - r[0] + 1) > WIDE:
                        m = _hull2(l, r)
                        fn(0, 128, m[0], m[1])
                        return
                    fn(0, 64, l[0], l[1])
                    fn(64, 128, r[0], r[1])
                elif l is not None:
                    fn(0, 64, l[0], l[1])
                else:
                    fn(64, 128, r[0], r[1])

            for it in range(15):
                y0 = max(0, 16 * it - 8)
                y1 = min(OUT, 16 * it + 8)
                H = y1 - y0
                J0 = 16 * it - 14
                itile = it_bufs[it % 2]
                P = _band_plan(y0, y1)
                has14 = any(h for h in P["h14"])
                has20 = any(h for h in P["h20"])
                has8 = any(h for h in P["h8"])
                has2 = any(h for h in P["h2"])
                need_oo = has20 or has8

                # ---- input DMA: one 16KB-contiguous chunk per partition ----
                if it < 14:
                    r0 = 32 * it
                    nc.sync.dma_start(itile[:, :, :], x[:, r0:r0 + 32, :])

                # ---- Ew/Ow rolling tiles ----
                if it == 0:
                    nc.gpsimd.memset(ewt[:, 0:28, :], NEG)
                    nc.gpsimd.memset(owt[:, 0:28, :], NEG)
                else:
                    # rolling ee/oo only read ewt rows 24.. / owt rows 25..
                    act.copy(ewt[:, 24:28, :], ewt[:, 56:60, :])
                    act.copy(owt[:, 25:28, :], owt[:, 57:60, :])
                if it < 14:
                    # host de-interleaved: A=T[1::2] at cols 0:125,
                    # B=T[2::2] at cols 125:249.  Ew[e]=max(A[e],B[e]),
                    # Ow[e]=max(B[e],A[e+1]) — all step-1 reads (2x mode).
                    dve.tensor_tensor(ewt[:, 28:60, :],
                                      itile[:, :, 0:124],
                                      itile[:, :, 125:249], mx)
                    gps.tensor_tensor(owt[:, 28:60, :],
                                      itile[:, :, 125:249],
                                      itile[:, :, 1:125], mx)
                else:
                    nc.gpsimd.memset(ewt[:, 28:60, :], NEG)
                    nc.gpsimd.memset(owt[:, 28:60, :], NEG)

                # ---- EE / OO (rolling: 12 halo rows rolled, 16 fresh) ----
                if it == 0:
                    nc.gpsimd.memset(ee[:, 0:12, :], NEG)
                    nc.gpsimd.memset(oo[:, 0:12, :], NEG)
                else:
                    act.copy(ee[:, 0:12, :], ee[:, 16:28, :])
                    act.copy(oo[:, 0:12, :], oo[:, 16:28, :])
                dve.tensor_tensor(ee[:, 12:28, :],
                                  ewt[:, 24:56:2, :],
                                  ewt[:, 25:57:2, :], mx)
                gps.tensor_tensor(oo[:, 12:28, :],
                                  owt[:, 25:57:2, :],
                                  owt[:, 26:58:2, :], mx)

                # ---- gated combine helpers ----
                def rows(tile_base, lo, hi):
                    return lo - tile_base, hi - tile_base

                def g_rcomb(dst, dst_base, src, src_base, jlo, jhi, d, ivs):
                    a, b = rows(src_base, jlo, jhi)
                    o0, o1 = rows(dst_base, jlo, jhi)

                    def f(p0, p1, lo, hi):
                        dve.tensor_tensor(dst[p0:p1, o0:o1, lo:hi + 1],
                                          src[p0:p1, a:b, lo:hi + 1],
                                          src[p0:p1, a + d:b + d, lo:hi + 1],
                                          mx)
                    emit(f, ivs)

                def g_ccomb(dst, src, nrows, d, ivs):
                    def f(p0, p1, lo, hi):
                        dve.tensor_tensor(dst[p0:p1, 0:nrows, lo:hi + 1],
                                          src[p0:p1, 0:nrows, lo:hi + 1],
                                          src[p0:p1, 0:nrows, lo + d:hi + 1 + d],
                                          mx)
                    emit(f, ivs)

                # ---- shared S2/S4 stages: full-width rolling 2D tiles ----
                FULL = [(0, 123), (0, 123)]
                if it == 0:
                    a2 = tP.tile([128, 27, EW], DT, tag="p0")
                    g_rcomb(a2, y0 - 6, ee, J0, y0 - 6, y1 + 5, 1, FULL)
                    g_ccomb(s2t, a2, H + 11, 1, [(0, 122)] * 2)
                    a4 = tP.tile([128, 27, EW], DT, tag="p0")
                    g_rcomb(a4, y0 - 6, s2t, y0 - 6, y0 - 6, y1 + 3, 2,
                            [(0, 122)] * 2)
                    g_ccomb(s4t, a4, H + 9, 2, [(0, 120)] * 2)
                    a2o = tPo.tile([128, 24, EW], DT, tag="po")
                    g_rcomb(a2o, y0 - 5, oo, J0, y0 - 5, y1 + 3, 1, FULL)
                    g_ccomb(s2ot, a2o, H + 8, 1, [(0, 122)] * 2)
                    a4o = tPo.tile([128, 24, EW], DT, tag="po")
                    g_rcomb(a4o, y0 - 5, s2ot, y0 - 5, y0 - 5, y1 + 1, 2,
                            [(0, 122)] * 2)
                    g_ccomb(s4o, a4o, H + 6, 2, [(0, 120)] * 2)
                else:
                    so = 8 if it == 1 else 16
                    act.copy(s2t[:, 0:11, :], s2t[:, so:so + 11, :])
                    act.copy(s4t[:, 0:9, :], s4t[:, so:so + 9, :])
                    act.copy(s2ot[:, 0:8, :], s2ot[:, so:so + 8, :])
                    act.copy(s4o[:, 0:6, :], s4o[:, so:so + 6, :])
                    a2f = tP.tile([128, 16, EW], DT, tag="p0")
                    dve.tensor_tensor(a2f[:, 0:H, :], ee[:, 11:11 + H, :],
                                      ee[:, 12:12 + H, :], mx)
                    dve.tensor_tensor(s2t[:, 11:11 + H, 0:123],
                                      a2f[:, 0:H, 0:123],
                                      a2f[:, 0:H, 1:124], mx)
                    a4f = tP.tile([128, 16, EW], DT, tag="p0")
                    dve.tensor_tensor(a4f[:, 0:H, 0:123],
                                      s2t[:, 9:9 + H, 0:123],
                                      s2t[:, 11:11 + H, 0:123], mx)
                    dve.tensor_tensor(s4t[:, 9:9 + H, 0:121],
                                      a4f[:, 0:H, 0:121],
                                      a4f[:, 0:H, 2:123], mx)
                    a2of = tPo.tile([128, 16, EW], DT, tag="po")
                    dve.tensor_tensor(a2of[:, 0:H, :], oo[:, 9:9 + H, :],
                                      oo[:, 10:10 + H, :], mx)
                    dve.tensor_tensor(s2ot[:, 8:8 + H, 0:123],
                                      a2of[:, 0:H, 0:123],
                                      a2of[:, 0:H, 1:124], mx)
                    a4of = tPo.tile([128, 16, EW], DT, tag="po")
                    dve.tensor_tensor(a4of[:, 0:H, 0:123],
                                      s2ot[:, 6:6 + H, 0:123],
                                      s2ot[:, 8:8 + H, 0:123], mx)
                    dve.tensor_tensor(s4o[:, 6:6 + H, 0:121],
                                      a4of[:, 0:H, 0:121],
                                      a4of[:, 0:H, 2:123], mx)

                # ---- EE-side deep pyramid (band-local, gated) ----
                a8 = tP.tile([128, 27, EW], DT, tag="p0")
                g_rcomb(a8, y0 - 6, s4t, y0 - 6, y0 - 6, y1 - 1, 4, P["a8"])
                if has14:
                    u = tR.tile([128, 16, EW], DT, tag="r0")
                    g_rcomb(u, y0 - 3, s4t, y0 - 6, y0 - 3, y1 - 3, 3, P["u"])
                    s7 = tR.tile([128, 16, EW], DT, tag="r0")
                    g_ccomb(s7, u, H, 3, P["s7"])
                s8 = tQ.tile([128, 27, EW], DT, tag="q0")
                g_ccomb(s8, a8, H + 5, 4, P["v"])
                v = tR.tile([128, 16, EW], DT, tag="r0")
                g_rcomb(v, y0 - 6, s8, y0 - 6, y0 - 6, y1 - 6, 5, P["v"])
                s13 = tR.tile([128, 16, OW], DT, tag="r13", bufs=1)
                g_ccomb(s13, v, H, 5, P["s13"])

                # ---- OO-side deep pyramid (band-local, gated) ----
                if has20:
                    a8o = tPo.tile([128, 24, EW], DT, tag="po")
                    g_rcomb(a8o, y0 - 5, s4o, y0 - 5, y0 - 5, y1 - 3, 4,
                            P["a8o"])
                    s8o = tQo.tile([128, 24, EW], DT, tag="qo")
                    g_ccomb(s8o, a8o, H + 2, 4, P["w"])
                    w = tRo.tile([128, 16, EW], DT, tag="ro")
                    g_rcomb(w, y0 - 5, s8o, y0 - 5, y0 - 5, y1 - 5, 2, P["w"])
                    s10 = tRo.tile([128, 16, EW], DT, tag="ro")
                    g_ccomb(s10, w, H, 2, P["s10"])

                # ---- masks DMA ----
                nc.sync.dma_start(mask_t[:, :, 0:H, :], rmask[:, :, y0:y1, :])

                # ---- blend: disjoint annulus masks over ring hulls ----
                def g_cp(ridx, srct, delta, ivs, srow0=0):
                    def f(p0, p1, lo, hi):
                        dve.copy_predicated(
                            s13[p0:p1, 0:H, lo:hi + 1],
                            mask_t[p0:p1, ridx, 0:H, lo:hi + 1],
                            srct[p0:p1, srow0:srow0 + H,
                                 lo + delta:hi + 1 + delta])
                    emit(f, ivs)

                # disk (r2) first as an unconditional 4x-rate copy: its hull
                # rect corners are provably inside r8/r14, whose predicated
                # copies run after and fix them up.
                if has2:
                    # per-half only: a merged rect would span outer-ring
                    # columns between the two disk hulls, which nothing fixes.
                    for wg, iv in enumerate(P["h2"]):
                        if iv:
                            dve.tensor_copy(
                                s13[64 * wg:64 * wg + 64, 0:H,
                                    iv[0]:iv[1] + 1],
                                ee[64 * wg:64 * wg + 64,
                                   y0 - J0:y0 - J0 + H,
                                   iv[0] + 6:iv[1] + 7])
                if has20:
                    g_cp(0, s10, 1, P["h20"])
                if has14:
                    g_cp(1, s7, 3, P["h14"])
                if has8:
                    g_cp(2, s4o, 4, P["h8"], srow0=3)

                # ---- store (fp16, host reassembles + casts) ----
                nc.sync.dma_start(y[:, y0:y1, :], s13[:, 0:H, :])

    return nc


def _get_nc():
    if "nc" not in _CACHED:
        nc = bass.Bass()
        _emit_kernel(nc)
        split_multi_waits(nc)
        _CACHED["nc"] = nc
    return _CACHED["nc"]


def make_in_maps(x: np.ndarray) -> list:
    """Per-core per-partition fp16 input with NEG pads baked in, even/odd
    columns de-interleaved (A=T[1::2] -> cols 0:125, B=T[2::2] -> 125:249)."""
    x16 = np.asarray(x, dtype=np.float32).astype(np.float16)
    in_maps = []
    for b in range(B):
        raw = np.full((128, IN, WIN), NEG, np.float16)
        raw[0:64, :, 13:250] = x16[b, :, :, 0:237]
        raw[64:128, :, 1:237] = x16[b, :, :, 212:448]
        xp = np.full((128, IN, WIN), NEG, np.float16)
        xp[:, :, 0:125] = raw[:, :, 1:250:2]
        xp[:, :, 125:249] = raw[:, :, 2:250:2]
        in_maps.append({"x": np.ascontiguousarray(xp)})
    return in_maps


def kernel(x: np.ndarray) -> np.ndarray:
    nc = _get_nc()
    in_maps = make_in_maps(x)
    res = run_bass_kernel_spmd(nc, in_maps, core_ids=list(range(B)))
    out = np.empty((B, C, OUT, OUT), np.float32)
    for b, r in enumerate(res.results):
        yp = r["y"]  # [128, 224, 112] fp16
        out[b, :, :, 0:OW] = yp[0:64].astype(np.float32)
        out[b, :, :, OW:OUT] = yp[64:128].astype(np.float32)
    return out
